# revision 28
# baseline (speedup 1.0000x reference)
"""nn_AttentionOnDetail Trainium2 Bass kernel, 8 NeuronCores.  v2

Sharding: stage 1 (AFT) is T-sharded (each core owns 256 timesteps of both
batches); 4 quarter-size fp16 AllToAlls re-shard to head-parallel (each
core owns 2 heads of 16) for the causal SDPA; 2 fp16 AllToAlls re-shard
back to T for the output projection.  Per-core stage-1 layout is
(128 partitions, 4 row-tiles, 1024 ch), row r = rt*128 + p
(rt 0,1 = batch 0 t-halves; rt 2,3 = batch 1).  Stage-1 is processed in
two chunks c = {rt, rt+2} (the AFT reduction pairs rt with rt+2) so the
first A2A fires at the half-way point.

Math notes:
 - xs = 2*pi*sigmoid(x) - pi == pi*tanh(x/2); harmonics sin/cos(n*pi*h)
   built from s=sin(pi*h), c1=1-2*sin(pi*h/2)^2 via Chebyshev products.
 - The 3x8 harmonic combination runs on TensorEngine as fp16 matmuls with
   scaled-identity stationary weights (PSUM accumulates); +K bias applied
   during the PSUM->SBUF writeback.
 - Stage-2 rotary is skipped entirely: q == k and the rotation is
   orthogonal per head, so it cancels in q @ k^T.
 - SDPA scores are computed transposed (S^T[k,q]); the two heads' score
   matmuls use partition bases 0/64 -> concurrent PE row-groups.  Causal
   diagonal tiles are width-trimmed (only cols >= 128*(kt-4qc) computed)
   and the residual triangle masked by a uniform gpsimd affine_select on
   the leading 128 columns.  V' = [V | 1] gives softmax denominators free.
 - Activation-table discipline: all Sin ops (both chunks) issue before the
   first Exp, so ACT swaps tables exactly once (silu -> exp set).
"""
import sys
import numpy as np

sys.path.insert(0, "/opt/trn_rl_repo")

import concourse.bass as bass
import concourse.mybir as mybir
import concourse.tile as tile
from concourse import bacc
from concourse.bass_utils import run_bass_kernel_spmd
from concourse.masks import make_identity

F32 = mybir.dt.float32
F32R = mybir.dt.float32r
F16 = mybir.dt.float16
BF16 = mybir.dt.bfloat16
I16 = mybir.dt.int16
I32 = mybir.dt.int32
AF = mybir.ActivationFunctionType
ALU = mybir.AluOpType

B, T, C, H, HD = 2, 2048, 1024, 16, 64
NCORES = 8
TSH = T // NCORES            # 256 timesteps per core
EPS = 1.1920929e-07
SDPA_SCALE = 0.12
PI = float(np.pi)

_CACHE = {}


def _quake_rsqrt(nc, pool, m_ap, n, tag, name=None):
    """rfac = 1/sqrt(m) for m (128, n) positive fp32, on DVE only."""
    name = name or tag
    sh = [128, n]
    it = pool.tile(sh, I32, tag=f"{tag}_i", name=f"{name}_i")
    nc.vector.tensor_scalar(it[:], m_ap.bitcast(I32), 1, None,
                            ALU.logical_shift_right)
    sd = pool.tile(sh, I32, tag=f"{tag}_s", name=f"{name}_s")
    nc.vector.tensor_scalar(sd[:], it[:], -1, 0x5F3759DF, ALU.mult, ALU.add)
    y0 = sd[:].bitcast(F32)
    t = pool.tile(sh, F32, tag=f"{tag}_t", name=f"{name}_t")
    nc.vector.tensor_tensor(t[:], y0, y0, ALU.mult)
    nc.vector.tensor_tensor(t[:], t[:], m_ap, ALU.mult)
    nc.vector.tensor_scalar(t[:], t[:], -0.5, 1.5, ALU.mult, ALU.add)
    nc.vector.tensor_tensor(t[:], t[:], y0, ALU.mult)       # y1
    y2 = pool.tile(sh, F32, tag=f"{tag}_y", name=f"{name}_y")
    nc.vector.tensor_tensor(y2[:], t[:], t[:], ALU.mult)
    nc.vector.tensor_tensor(y2[:], y2[:], m_ap, ALU.mult)
    nc.vector.tensor_scalar(y2[:], y2[:], -0.5, 1.5, ALU.mult, ALU.add)
    nc.vector.tensor_tensor(y2[:], y2[:], t[:], ALU.mult)
    return y2


def build():
    nc = bacc.Bacc("TRN2", target_bir_lowering=False, debug=False,
                   num_devices=NCORES)
    xs_d = nc.dram_tensor("xs", [128, 4, 1024], F32, kind="ExternalInput")
    combw_d = nc.dram_tensor("combw", [128, 24, 128], F16,
                             kind="ExternalInput")
    kvec_d = nc.dram_tensor("kvec", [128, 4], F32, kind="ExternalInput")
    rotc1_d = nc.dram_tensor("rotc1", [128, 16, 16], F16,
                             kind="ExternalInput")
    rots1_d = nc.dram_tensor("rots1", [128, 16, 16], F16,
                             kind="ExternalInput")
    aftT_d = nc.dram_tensor("aftT", [128, 8, 1024], F16,
                            kind="ExternalInput")
    mhaT_d = nc.dram_tensor("mhaT", [128, 8, 1024], F16,
                            kind="ExternalInput")
    out_d = nc.dram_tensor("out", [128, 4, 1024], F32, kind="ExternalOutput")

    with tile.TileContext(nc) as tc:
      with tc.tile_pool(name="glob", bufs=1) as gp, \
           tc.tile_pool(name="projp", bufs=1) as pw, \
           tc.tile_pool(name="dram", bufs=1, space="DRAM") as dpool:

        # ---- DRAM bounce buffers for the collectives -------------------
        a2a1_in = [dpool.tile([NCORES, 2, 128, 128], F16,
                              name=f"a2a1_in{b}") for b in range(2)]
        a2a1_out = [dpool.tile([NCORES, 2, 128, 128], F16,
                               name=f"a2a1_out{b}") for b in range(2)]
        a2a2_in = [dpool.tile([NCORES, 2, 64, 256], F16,
                              name=f"a2a2_in{b}") for b in range(2)]
        a2a2_out = [dpool.tile([NCORES, 2, 64, 256], F16,
                               name=f"a2a2_out{b}") for b in range(2)]
        dum_in = dpool.tile([8, 4], F32, name="dum_in")
        dum_out = dpool.tile([8, 4], F32, name="dum_out")

        # ---- input DMAs (x per chunk so fe(c0) starts early) -----------
        x = gp.tile([128, 4, 1024], F32, tag="x", name="x")
        for c in range(2):
            nc.sync.dma_start(out=x[:, c:c + 3:2, :],
                              in_=xs_d[:, c:c + 3:2, :])
        kvec = gp.tile([128, 4], F32, name="kvec")
        nc.sync.dma_start(out=kvec[:], in_=kvec_d[:])
        nc.sync.dma_start(out=dum_in[:], in_=kvec_d[0:8, :])
        nc.gpsimd.collective_compute(
            "AllToAll", ALU.bypass,
            replica_groups=[list(range(NCORES))],
            ins=[dum_in[:].opt()], outs=[dum_out[:].opt()])
        combw = gp.tile([128, 24, 128], F16, name="combw")
        nc.sync.dma_start(out=combw[:], in_=combw_d[:])
        rotc1 = gp.tile([128, 16, 16], F16, name="rotc1")
        rots1 = gp.tile([128, 16, 16], F16, name="rots1")
        nc.sync.dma_start(out=rotc1[:], in_=rotc1_d[:])
        nc.sync.dma_start(out=rots1[:], in_=rots1_d[:])
        aftw = pw.tile([128, 8, 1024], F16, tag="projw", name="aftw")
        nc.sync.dma_start(out=aftw[:], in_=aftT_d[:])
        ident = gp.tile([128, 128], F16, name="ident")
        make_identity(nc, ident[:])

        qkv = [[gp.tile([128, 2, 1024], F16, tag=f"qkv{i}_{c}",
                        name=f"qkv{i}_{c}") for i in range(3)]
               for c in range(2)]
        a_ch = [gp.tile([128, 2, 1024], F16, tag=f"a_{c}", name=f"a_{c}")
                for c in range(2)]

        BAS = ("sA", "c1", "m", "p_", "sp", "cp", "mp", "mm")
        basis = [dict() for _ in range(2)]

        # ================= stage 1 (both chunks) ========================
        with tc.tile_pool(name="fe", bufs=2) as fep, \
             tc.tile_pool(name="psA", bufs=2, space="PSUM") as psA, \
             tc.tile_pool(name="pB", bufs=2) as pB, \
             tc.tile_pool(name="psT", bufs=1, space="PSUM") as psT, \
             tc.tile_pool(name="pB1", bufs=1) as pB1, \
             tc.tile_pool(name="psP", bufs=1, space="PSUM") as psP:

            # ---- phase A: front-end basis + combine, both chunks -------
            for c in range(2):
                xc = x[:, c:c + 3:2, :]                       # (128,2,1024)
                h = fep.tile([128, 2, 1024], F16, tag="h", name=f"h{c}")
                nc.scalar.activation(h[:], xc, AF.Tanh, scale=0.5)
                bs = {nm: fep.tile([128, 2, 1024], F16, tag=nm,
                                   name=f"{nm}{c}") for nm in BAS}
                basis[c] = bs
                nc.scalar.activation(bs["sA"][:], h[:], AF.Sin, scale=PI)
                sB = fep.tile([128, 2, 1024], F16, tag="sB", name=f"sB{c}")
                nc.scalar.activation(sB[:], h[:], AF.Sin, scale=PI / 2)
                nc.scalar.activation(sB[:], sB[:], AF.Square)        # u
                nc.vector.tensor_scalar(bs["c1"][:], sB[:], -2.0, 1.0,
                                        ALU.mult, ALU.add)
                nc.scalar.activation(bs["p_"][:], bs["sA"][:], AF.Square)
                nc.vector.tensor_tensor(bs["m"][:], bs["sA"][:],
                                        bs["c1"][:], ALU.mult)
                nc.vector.tensor_tensor(bs["sp"][:], bs["sA"][:],
                                        bs["p_"][:], ALU.mult)
                nc.vector.tensor_tensor(bs["cp"][:], bs["c1"][:],
                                        bs["p_"][:], ALU.mult)
                nc.vector.tensor_tensor(bs["mp"][:], bs["m"][:],
                                        bs["p_"][:], ALU.mult)
                nc.scalar.activation(bs["mm"][:], bs["m"][:], AF.Square)

                for pc in range(4):
                    rs, c0 = pc // 2, 512 * (pc % 2)
                    pss = [psA.tile([128, 512], F32, tag=f"c{i}",
                                    name=f"c{i}") for i in range(3)]
                    for f in range(8):
                        mv = bs[BAS[f]][:, rs, c0:c0 + 512]
                        for i in range(3):
                            nc.tensor.matmul(
                                pss[i][:], combw[:, 8 * i + f, :], mv,
                                start=(f == 0), stop=(f == 7))
                    dst = [qkv[c][i][:, rs, c0:c0 + 512] for i in range(3)]
                    nc.scalar.activation(dst[0], pss[0][:], AF.Identity,
                                         bias=kvec[:, 0:1])
                    if c == 0:
                        nc.vector.tensor_scalar(dst[1], pss[1][:],
                                                kvec[:, 1:2], None, ALU.add)
                    else:
                        nc.scalar.activation(dst[1], pss[1][:], AF.Identity,
                                             bias=kvec[:, 1:2])
                    nc.scalar.activation(dst[2], pss[2][:], AF.Identity,
                                         bias=kvec[:, 2:3])

            def _warm(n, tag):
                jp = psP.tile([128, 512], F32, tag="pa",
                              name=f"warm{tag}")
                for j in range(n):
                    nc.tensor.matmul(jp[:], combw[:, j % 24, :],
                                     aftw[:, j % 8, 0:512],
                                     start=(j == 0), stop=(j == n - 1))
            _warm(0, "a")

            # ---- phase B+C: rms+rotary+AFT, proj, A2A#1, per chunk -----
            for c in range(2):
                # Per tensor: stats (rotation-invariant) run concurrent
                # with the in-place rotary on the other engine; the rms
                # apply follows both (rot(x)*rf == rot(x*rf)).
                cb = rotc1[:].unsqueeze(1).broadcast_to([128, 2, 16, 16])
                sb_ = rots1[:].unsqueeze(1).broadcast_to([128, 2, 16, 16])
                sq = pB.tile([128, 2, 1024], F16, tag="sq", name=f"sq{c}")
                rfs = [None, None, None]

                def _stats(i):
                    if c == 0:
                        nc.vector.tensor_tensor(
                            sq[:].rearrange("p a b -> p (a b)"),
                            qkv[c][i][:].rearrange("p a b -> p (a b)"),
                            qkv[c][i][:].rearrange("p a b -> p (a b)"),
                            ALU.mult)
                    else:
                        nc.scalar.activation(
                            sq[:].rearrange("p a b -> p (a b)"),
                            qkv[c][i][:].rearrange("p a b -> p (a b)"),
                            AF.Square)
                    ssq = pB.tile([128, 32], F32, tag=f"ssq{i}",
                                  name=f"ssq{i}_{c}")
                    nc.vector.tensor_reduce(
                        ssq[:],
                        sq[:].rearrange("p a (h d) -> p (a h) d", h=16),
                        axis=mybir.AxisListType.X, op=ALU.add)
                    nc.vector.tensor_scalar(ssq[:], ssq[:], 1.0 / 64, EPS,
                                            ALU.mult, ALU.add)
                    rf = _quake_rsqrt(nc, pB, ssq[:], 32, f"rf{i}",
                                      f"rf{i}_{c}")
                    rf16 = pB.tile([128, 32], F16, tag=f"rh{i}",
                                   name=f"rh{i}_{c}")
                    nc.vector.tensor_copy(rf16[:], rf[:])
                    rfs[i] = rf16

                def _rot(i, eng):
                    qv = qkv[c][i][:].rearrange("p a (h d) -> p a h d",
                                                h=16)
                    x1 = qv[:, :, :, 0:16]
                    x2 = qv[:, :, :, 32:48]
                    u1 = pB.tile([128, 2, 16, 16], F16, tag="ru1",
                                 name=f"ru1{i}_{c}")
                    u2 = pB.tile([128, 2, 16, 16], F16, tag="ru2",
                                 name=f"ru2{i}_{c}")
                    t1 = pB.tile([128, 2, 16, 16], F16, tag="rt1",
                                 name=f"rt1{i}_{c}")
                    eng.tensor_tensor(u1[:], x2, sb_, ALU.mult)
                    eng.tensor_tensor(u2[:], x1, sb_, ALU.mult)
                    eng.tensor_tensor(t1[:], x1, cb, ALU.mult)
                    eng.tensor_tensor(x1, t1[:], u1[:], ALU.add)
                    eng.tensor_tensor(t1[:], x2, cb, ALU.mult)
                    eng.tensor_tensor(x2, t1[:], u2[:], ALU.subtract)

                def _apply(i, eng):
                    rb = rfs[i][:].rearrange("p (a h) -> p a h", a=2) \
                        .unsqueeze(3).broadcast_to([128, 2, 16, 64])
                    v4 = qkv[c][i][:].rearrange("p a (h d) -> p a h d",
                                                h=16)
                    eng.tensor_tensor(v4, v4, rb, ALU.mult)

                _stats(0)
                _stats(1)
                _stats(2)
                _apply(0, nc.vector)
                _apply(1, nc.gpsimd)
                _apply(2, nc.vector)
                _rot(0, nc.vector)
                _rot(1, nc.gpsimd)

                ek = qkv[c][1]
                nc.scalar.activation(ek[:].rearrange("p a b -> p (a b)"),
                                     ek[:].rearrange("p a b -> p (a b)"),
                                     AF.Exp)
                s_ = pB.tile([128, 1024], F32, tag="s_", name=f"s_{c}")
                nc.vector.tensor_tensor(s_[:], ek[:, 0, :], ek[:, 1, :],
                                        ALU.add)
                sinv = pB1.tile([128, 1024], F32, tag="sinv",
                               name=f"sinv{c}")
                nc.vector.reciprocal_approx_fast(sinv[:], s_[:])
                t0 = pB.tile([128, 1024], F16, tag="t0", name=f"t0{c}")
                t1_ = pB.tile([128, 1024], F16, tag="t1", name=f"t1{c}")
                nc.vector.tensor_tensor(t0[:], ek[:, 0, :],
                                        qkv[c][2][:, 0, :], ALU.mult)
                nc.gpsimd.tensor_tensor(t1_[:], ek[:, 1, :],
                                        qkv[c][2][:, 1, :], ALU.mult)
                nc.vector.tensor_tensor(t0[:], t0[:], t1_[:], ALU.add)
                r_ = pB.tile([128, 1024], F16, tag="r_", name=f"r_{c}")
                nc.vector.tensor_tensor(r_[:], t0[:], sinv[:], ALU.mult)
                tq = qkv[c][0]
                nc.scalar.activation(tq[:].rearrange("p a b -> p (a b)"),
                                     tq[:].rearrange("p a b -> p (a b)"),
                                     AF.Tanh, scale=0.5)
                y1 = qkv[c][2]
                rb_ = r_[:].unsqueeze(1).broadcast_to([128, 2, 1024])
                nc.vector.scalar_tensor_tensor(
                    y1[:], tq[:], 1.0, rb_, ALU.add, ALU.mult)

                # transpose y1 -> (c-part, t-free), aft projection
                y1T = pB.tile([128, 8, 256], F16, tag="y1T", name=f"y1T{c}")
                for rs in range(2):
                    for cp2 in range(4):
                        pst = psT.tile([128, 256], F16, tag="pst",
                                       name="pst")
                        for k2 in range(2):
                            cb8 = 2 * cp2 + k2
                            nc.tensor.transpose(
                                pst[:, 128 * k2:128 * (k2 + 1)],
                                y1[:, rs, 128 * cb8:128 * (cb8 + 1)],
                                ident[:])
                        nc.any.tensor_copy(
                            y1T[:, 2 * cp2:2 * cp2 + 2,
                                128 * rs:128 * (rs + 1)],
                            pst[:].rearrange("p (a b) -> p a b", a=2))
                for rs in range(2):
                    for oc in range(2):
                        pa = psP.tile([128, 512], F32, tag="pa", name="pa")
                        for cb8 in range(8):
                            nc.tensor.matmul(
                                pa[:],
                                y1T[:, cb8, 128 * rs:128 * (rs + 1)],
                                aftw[:, cb8, 512 * oc:512 * (oc + 1)],
                                start=(cb8 == 0), stop=(cb8 == 7))
                        nc.any.tensor_copy(
                            a_ch[c][:, rs, 512 * oc:512 * (oc + 1)],
                            pa[:])
                for b in range(2):
                    nc.sync.dma_start(
                        out=a2a1_in[b][:, c, :, :].rearrange(
                            "d p c2 -> p d c2"),
                        in_=a_ch[c][:, b, :].rearrange(
                            "p (d c2) -> p d c2", d=8))
                _warm(0, f"c{c}")
            nc.gpsimd.collective_compute(
                "AllToAll", ALU.bypass,
                replica_groups=[list(range(NCORES))],
                ins=[dum_in[:].opt()], outs=[dum_out[:].opt()])
            for b in range(2):
                nc.gpsimd.collective_compute(
                    "AllToAll", ALU.bypass,
                    replica_groups=[list(range(NCORES))],
                    ins=[a2a1_in[b][:].opt()],
                    outs=[a2a1_out[b][:].opt()])

        # ============ stage 2: causal SDPA + stage 3 projection =========
        with tc.tile_pool(name="pE", bufs=1) as pE, \
             tc.tile_pool(name="psE", bufs=2, space="PSUM") as psE, \
             tc.tile_pool(name="psY", bufs=1, space="PSUM") as psY, \
             tc.tile_pool(name="psQ", bufs=1, space="PSUM") as psQ, \
             tc.tile_pool(name="pe16", bufs=6) as pe16, \
             tc.tile_pool(name="pH", bufs=1) as pH, \
             tc.tile_pool(name="psH", bufs=1, space="PSUM") as psH:
            out_sb = pH.tile([128, 4, 1024], F32, tag="out_sb",
                             name="out_sb")
            mhaw = pH.tile([128, 8, 1024], F16, tag="mhaw", name="mhaw")
            nc.scalar.dma_start(out=mhaw[:], in_=mhaT_d[:])
            for b in range(2):
                A2 = pE.tile([128, 16, 128], F16, tag=f"A2_{b}",
                             name=f"A2_{b}")
                nc.sync.dma_start(
                    out=A2[:],
                    in_=a2a1_out[b][:].rearrange(
                        "s c p c2 -> p (s c) c2"))
                sq2 = pE.tile([128, 16, 128], F32, tag="sq2",
                              name=f"sq2_{b}")
                nc.scalar.activation(
                    sq2[:].rearrange("p s c2 -> p (s c2)"),
                    A2[:].rearrange("p s c2 -> p (s c2)"), AF.Square)
                ssq2 = pE.tile([128, 32], F32, tag="ssq2", name=f"ssq2{b}")
                nc.vector.tensor_reduce(
                    ssq2[:],
                    sq2[:].rearrange("p s (h d) -> p (s h) d", h=2),
                    axis=mybir.AxisListType.X, op=ALU.add)
                nc.vector.tensor_scalar(ssq2[:], ssq2[:], 1.0 / 64, EPS,
                                        ALU.mult, ALU.add)
                rf2 = _quake_rsqrt(nc, pE, ssq2[:], 32, f"rf2_{b}")
                rf216 = pE.tile([128, 32], F16, tag="rf216",
                                name=f"rf216_{b}")
                nc.vector.tensor_copy(rf216[:], rf2[:])
                V16 = pE.tile([128, 16, 2, 65], BF16, tag=f"V16_{b}",
                              name=f"V16_{b}")
                A16 = pE.tile([128, 16, 128], F16, tag=f"A16_{b}",
                              name=f"A16_{b}")
                rb2 = rf216[:].rearrange("p (s h) -> p s h", s=16) \
                    .unsqueeze(3).broadcast_to([128, 16, 2, 64])
                nc.vector.tensor_tensor(
                    A16[:].rearrange("p s (h d) -> p s h d", h=2),
                    A2[:].rearrange("p s (h d) -> p s h d", h=2),
                    rb2, ALU.mult)
                nc.vector.tensor_copy(
                    V16[:, :, :, 0:64],
                    A16[:].rearrange("p s (h d) -> p s h d", h=2))
                nc.vector.memset(V16[:, :, :, 64:65], 1.0)

                QT = pE.tile([128, 2048], F16, tag=f"QT_{b}",
                             name=f"QT_{b}")
                for jq in range(4):
                    pst2 = psQ.tile([128, 512], F16, tag="pst2",
                                    name="pst2")
                    for k4 in range(4):
                        j = 4 * jq + k4
                        nc.tensor.transpose(
                            pst2[:, 128 * k4:128 * (k4 + 1)],
                            A16[:, j, :], ident[:])
                    nc.any.tensor_copy(QT[:, 512 * jq:512 * (jq + 1)],
                                       pst2[:])

                Y16 = pE.tile([64, 8, 512], F16, tag=f"Y16_{b}",
                              name=f"Y16_{b}")
                for qc in range(4):
                    pys = [psY.tile([65, 512], F32, tag=f"py{hh}",
                                    name=f"py{hh}") for hh in range(2)]
                    nkt = 4 * qc + 4
                    for kt in range(nkt):
                        off = max(0, 128 * (kt - 4 * qc))
                        ps_s = psE.tile([128, 2, 512], F32, tag="ps_s",
                                        name="ps_s")
                        for hh in range(2):
                            hb = 64 * hh
                            nc.tensor.matmul(
                                ps_s[:, hh, off:512],
                                QT[hb:hb + 64, 128 * kt:128 * (kt + 1)],
                                QT[hb:hb + 64, 512 * qc + off:
                                   512 * (qc + 1)],
                                start=True, stop=True)
                        e16 = pe16.tile([128, 2, 512], BF16, tag="e16",
                                        name="e16")
                        if kt % 3 == 2:
                            # Schraudolph exp in bf16 bit-space on DVE:
                            # bits = round(S*0.12*(2^7/ln2) + 127*128 - 11.5)
                            nc.vector.tensor_scalar(
                                e16[:, :, off:512].bitcast(I16),
                                ps_s[:, :, off:512],
                                22.159803, 16245.5, ALU.mult, ALU.add)
                        else:
                            nc.scalar.activation(e16[:, :, off:512],
                                                 ps_s[:, :, off:512], AF.Exp,
                                                 scale=SDPA_SCALE)
                        if kt >= 4 * qc:
                            for hh in range(2):
                                eh = e16[:, hh, off:off + 128]
                                nc.gpsimd.affine_select(
                                    out=eh, in_=eh,
                                    compare_op=ALU.is_ge, fill=0.0,
                                    base=0, pattern=[[1, 128]],
                                    channel_multiplier=-1)
                        for hh in range(2):
                            nc.tensor.matmul(
                                pys[hh][:, off:512],
                                V16[:, kt, hh, :],
                                e16[:, hh, off:512],
                                start=(kt == 0), stop=(kt == nkt - 1))
                    for hh in range(2):
                        den = pE.tile([1, 512], F32, tag="den",
                                      name=f"den{b}{qc}{hh}")
                        nc.any.tensor_copy(den[:], pys[hh][64:65, :])
                        rinv = pE.tile([1, 512], F32, tag="rinv",
                                       name=f"rinv{b}{qc}{hh}")
                        nc.vector.reciprocal_approx_fast(rinv[:], den[:])
                        RSR = pE.tile([64, 512], F32, tag=f"RSR{hh}",
                                      name=f"RSR{hh}")
                        nc.gpsimd.partition_broadcast(RSR[:], rinv[:])
                        nc.vector.tensor_tensor(
                            Y16[:, 4 * hh + qc, :], pys[hh][0:64, :],
                            RSR[:], ALU.mult)
                        nc.sync.dma_start(
                            out=a2a2_in[b][2 * qc:2 * qc + 2, hh, :, :]
                                .rearrange("d p q -> p d q"),
                            in_=Y16[:, 4 * hh + qc, :].rearrange(
                                "p (d q) -> p d q", d=2))
                nc.gpsimd.collective_compute(
                    "AllToAll", ALU.bypass,
                    replica_groups=[list(range(NCORES))],
                    ins=[a2a2_in[b][:].opt()],
                    outs=[a2a2_out[b][:].opt()])

            # ---- stage 3: output projection, per batch -----------------
            for b in range(2):
                YF = pH.tile([128, 8, 256], F16, tag=f"YF{b}",
                             name=f"YF{b}")
                nc.scalar.dma_start(
                    out=YF[:],
                    in_=a2a2_out[b][:].rearrange(
                        "s hh p q -> (hh p) s q"))
                for rl in range(2):
                    rt = 2 * b + rl
                    for oc in range(2):
                        pm = psH.tile([128, 512], F32, tag="pm", name="pm")
                        for s_ in range(NCORES):
                            nc.tensor.matmul(
                                pm[:],
                                YF[:, s_, 128 * rl:128 * (rl + 1)],
                                mhaw[:, s_, 512 * oc:512 * (oc + 1)],
                                start=(s_ == 0), stop=(s_ == 7))
                        nc.any.tensor_copy(
                            out_sb[:, rt, 512 * oc:512 * (oc + 1)], pm[:])
                nc.scalar.dma_start(out=out_d[:, 2 * b:2 * b + 2, :],
                                    in_=out_sb[:, 2 * b:2 * b + 2, :])

    nc.compile()
    return nc


def _host_inputs(x, kqv, c_proj):
    """Build per-core input maps from the full problem inputs."""
    A = kqv[:, :5].astype(np.float64)     # sin coefs (col n)
    Bc = kqv[:, 5:].astype(np.float64)    # cos coefs
    coef = np.zeros((8, 3), np.float64)   # basis {s,c1,m,p,sp,cp,mp,mm}
    K = A[:, 0] + Bc[:, 0] + Bc[:, 2] + Bc[:, 4]
    coef[0] = A[:, 1] + 3.0 * A[:, 3]
    coef[1] = Bc[:, 1] + Bc[:, 3]
    coef[2] = 2.0 * A[:, 2] + 4.0 * A[:, 4]
    coef[3] = -2.0 * Bc[:, 2]
    coef[4] = -4.0 * A[:, 3]
    coef[5] = -4.0 * Bc[:, 3]
    coef[6] = -8.0 * A[:, 4]
    coef[7] = -8.0 * Bc[:, 4]

    eye = np.eye(128, dtype=np.float32)
    combw = np.zeros((128, 24, 128), np.float16)
    for i in range(3):
        for f in range(8):
            combw[:, 8 * i + f, :] = (eye * np.float32(coef[f, i])) \
                .astype(np.float16)
    kvec = np.zeros((128, 4), np.float32)
    kvec[:, :3] = K.astype(np.float32)[None, :]

    freq = (1.0 / 1024.0) ** np.linspace(0.0, 1.0, 16, dtype=np.float32)
    hh = np.arange(16, dtype=np.float32)
    theta = np.outer(hh, freq)                       # (16 heads, 16 j)
    rotc1 = np.broadcast_to(np.cos(theta).astype(np.float16),
                            (128, 16, 16)).copy()
    rots1 = np.broadcast_to(np.sin(theta).astype(np.float16),
                            (128, 16, 16)).copy()

    W1 = c_proj[:, :C]
    W2 = c_proj[:, C:]
    aftT = (0.5 * W1.T).reshape(8, 128, 1024).transpose(1, 0, 2) \
        .astype(np.float16).copy()
    mhaT = W2.T.reshape(8, 128, 1024).transpose(1, 0, 2) \
        .astype(np.float16).copy()

    in_maps = []
    for c in range(NCORES):
        xs = x[:, TSH * c:TSH * (c + 1), :].reshape(4, 128, 1024) \
            .transpose(1, 0, 2).copy()
        in_maps.append(dict(xs=np.ascontiguousarray(xs), combw=combw,
                            kvec=kvec, rotc1=rotc1, rots1=rots1,
                            aftT=aftT, mhaT=mhaT))
    return in_maps


def kernel(x, kqv, c_proj):
    x = np.asarray(x, np.float32)
    kqv = np.asarray(kqv, np.float32)
    c_proj = np.asarray(c_proj, np.float32)
    if "nc" not in _CACHE:
        _CACHE["nc"] = build()
    nc = _CACHE["nc"]
    in_maps = _host_inputs(x, kqv, c_proj)
    res = run_bass_kernel_spmd(nc, in_maps, core_ids=list(range(NCORES)))
    out = np.empty((B, T, C), np.float32)
    for c in range(NCORES):
        oc = res.results[c]["out"]          # (128, 4, 1024)
        oc = oc.transpose(1, 0, 2).reshape(B, TSH, C)
        out[:, TSH * c:TSH * (c + 1), :] = oc
    return out


# revision 29
# speedup vs baseline: 1.1009x; 1.1009x over previous
"""nn_AttentionOnDetail Trainium2 Bass kernel, 8 NeuronCores.  v2

Sharding: stage 1 (AFT) is T-sharded (each core owns 256 timesteps of both
batches); 4 quarter-size fp16 AllToAlls re-shard to head-parallel (each
core owns 2 heads of 16) for the causal SDPA; 2 fp16 AllToAlls re-shard
back to T for the output projection.  Per-core stage-1 layout is
(128 partitions, 4 row-tiles, 1024 ch), row r = rt*128 + p
(rt 0,1 = batch 0 t-halves; rt 2,3 = batch 1).  Stage-1 is processed in
two chunks c = {rt, rt+2} (the AFT reduction pairs rt with rt+2) so the
first A2A fires at the half-way point.

Math notes:
 - xs = 2*pi*sigmoid(x) - pi == pi*tanh(x/2); harmonics sin/cos(n*pi*h)
   built from s=sin(pi*h), c1=1-2*sin(pi*h/2)^2 via Chebyshev products.
 - The 3x8 harmonic combination runs on TensorEngine as fp16 matmuls with
   scaled-identity stationary weights (PSUM accumulates); +K bias applied
   during the PSUM->SBUF writeback.
 - Stage-2 rotary is skipped entirely: q == k and the rotation is
   orthogonal per head, so it cancels in q @ k^T.
 - SDPA scores are computed transposed (S^T[k,q]); the two heads' score
   matmuls use partition bases 0/64 -> concurrent PE row-groups.  Causal
   diagonal tiles are width-trimmed (only cols >= 128*(kt-4qc) computed)
   and the residual triangle masked by a uniform gpsimd affine_select on
   the leading 128 columns.  V' = [V | 1] gives softmax denominators free.
 - Activation-table discipline: all Sin ops (both chunks) issue before the
   first Exp, so ACT swaps tables exactly once (silu -> exp set).
"""
import sys
import numpy as np

sys.path.insert(0, "/opt/trn_rl_repo")

import concourse.bass as bass
import concourse.mybir as mybir
import concourse.tile as tile
from concourse import bacc
from concourse.bass_utils import run_bass_kernel_spmd
from concourse.masks import make_identity

F32 = mybir.dt.float32
F32R = mybir.dt.float32r
F16 = mybir.dt.float16
BF16 = mybir.dt.bfloat16
I16 = mybir.dt.int16
I32 = mybir.dt.int32
AF = mybir.ActivationFunctionType
ALU = mybir.AluOpType

B, T, C, H, HD = 2, 2048, 1024, 16, 64
NCORES = 8
TSH = T // NCORES            # 256 timesteps per core
EPS = 1.1920929e-07
SDPA_SCALE = 0.12
PI = float(np.pi)

_CACHE = {}


def _quake_rsqrt(nc, pool, m_ap, n, tag, name=None):
    """rfac = 1/sqrt(m) for m (128, n) positive fp32, on DVE only."""
    name = name or tag
    sh = [128, n]
    it = pool.tile(sh, I32, tag=f"{tag}_i", name=f"{name}_i")
    nc.vector.tensor_scalar(it[:], m_ap.bitcast(I32), 1, None,
                            ALU.logical_shift_right)
    sd = pool.tile(sh, I32, tag=f"{tag}_s", name=f"{name}_s")
    nc.vector.tensor_scalar(sd[:], it[:], -1, 0x5F3759DF, ALU.mult, ALU.add)
    y0 = sd[:].bitcast(F32)
    t = pool.tile(sh, F32, tag=f"{tag}_t", name=f"{name}_t")
    nc.vector.tensor_tensor(t[:], y0, y0, ALU.mult)
    nc.vector.tensor_tensor(t[:], t[:], m_ap, ALU.mult)
    nc.vector.tensor_scalar(t[:], t[:], -0.5, 1.5, ALU.mult, ALU.add)
    nc.vector.tensor_tensor(t[:], t[:], y0, ALU.mult)       # y1
    y2 = pool.tile(sh, F32, tag=f"{tag}_y", name=f"{name}_y")
    nc.vector.tensor_tensor(y2[:], t[:], t[:], ALU.mult)
    nc.vector.tensor_tensor(y2[:], y2[:], m_ap, ALU.mult)
    nc.vector.tensor_scalar(y2[:], y2[:], -0.5, 1.5, ALU.mult, ALU.add)
    nc.vector.tensor_tensor(y2[:], y2[:], t[:], ALU.mult)
    return y2


def build():
    nc = bacc.Bacc("TRN2", target_bir_lowering=False, debug=False,
                   num_devices=NCORES)
    xs_d = nc.dram_tensor("xs", [128, 4, 1024], F32, kind="ExternalInput")
    combw_d = nc.dram_tensor("combw", [128, 24, 128], F16,
                             kind="ExternalInput")
    kvec_d = nc.dram_tensor("kvec", [128, 4], F32, kind="ExternalInput")
    rotc1_d = nc.dram_tensor("rotc1", [128, 16, 16], F16,
                             kind="ExternalInput")
    rots1_d = nc.dram_tensor("rots1", [128, 16, 16], F16,
                             kind="ExternalInput")
    aftT_d = nc.dram_tensor("aftT", [128, 8, 1024], F16,
                            kind="ExternalInput")
    mhaT_d = nc.dram_tensor("mhaT", [128, 8, 1024], F16,
                            kind="ExternalInput")
    out_d = nc.dram_tensor("out", [128, 4, 1024], F32, kind="ExternalOutput")

    with tile.TileContext(nc) as tc:
      with tc.tile_pool(name="glob", bufs=1) as gp, \
           tc.tile_pool(name="projp", bufs=1) as pw, \
           tc.tile_pool(name="dram", bufs=1, space="DRAM") as dpool:

        # ---- DRAM bounce buffers for the collectives -------------------
        a2a1_in = [dpool.tile([NCORES, 2, 128, 128], F16,
                              name=f"a2a1_in{b}") for b in range(2)]
        a2a1_out = [dpool.tile([NCORES, 2, 128, 128], F16,
                               name=f"a2a1_out{b}") for b in range(2)]
        a2a2_in = [dpool.tile([NCORES, 2, 64, 256], F16,
                              name=f"a2a2_in{b}") for b in range(2)]
        a2a2_out = [dpool.tile([NCORES, 2, 64, 256], F16,
                               name=f"a2a2_out{b}") for b in range(2)]
        dum_in = dpool.tile([8, 4], F32, name="dum_in")
        dum_out = dpool.tile([8, 4], F32, name="dum_out")

        # ---- input DMAs (x per chunk so fe(c0) starts early) -----------
        x = gp.tile([128, 4, 1024], F32, tag="x", name="x")
        for c in range(2):
            nc.sync.dma_start(out=x[:, c:c + 3:2, :],
                              in_=xs_d[:, c:c + 3:2, :])
        kvec = gp.tile([128, 4], F32, name="kvec")
        nc.sync.dma_start(out=kvec[:], in_=kvec_d[:])
        nc.sync.dma_start(out=dum_in[:], in_=kvec_d[0:8, :])
        nc.gpsimd.collective_compute(
            "AllToAll", ALU.bypass,
            replica_groups=[list(range(NCORES))],
            ins=[dum_in[:].opt()], outs=[dum_out[:].opt()])
        combw = gp.tile([128, 24, 128], F16, name="combw")
        nc.sync.dma_start(out=combw[:], in_=combw_d[:])
        rotc1 = gp.tile([128, 16, 16], F16, name="rotc1")
        rots1 = gp.tile([128, 16, 16], F16, name="rots1")
        nc.sync.dma_start(out=rotc1[:], in_=rotc1_d[:])
        nc.sync.dma_start(out=rots1[:], in_=rots1_d[:])
        aftw = pw.tile([128, 8, 1024], F16, tag="projw", name="aftw")
        nc.sync.dma_start(out=aftw[:], in_=aftT_d[:])
        ident = gp.tile([128, 128], F16, name="ident")
        make_identity(nc, ident[:])

        qkv = [[gp.tile([128, 2, 1024], F16, tag=f"qkv{i}_{c}",
                        name=f"qkv{i}_{c}") for i in range(3)]
               for c in range(2)]
        a_ch = [gp.tile([128, 2, 1024], F16, tag=f"a_{c}", name=f"a_{c}")
                for c in range(2)]

        BAS = ("sA", "c1", "m", "p_", "sp", "cp", "mp", "mm")
        basis = [dict() for _ in range(2)]

        # ================= stage 1 (both chunks) ========================
        with tc.tile_pool(name="fe", bufs=2) as fep, \
             tc.tile_pool(name="psA", bufs=2, space="PSUM") as psA, \
             tc.tile_pool(name="pB", bufs=2) as pB, \
             tc.tile_pool(name="psT", bufs=1, space="PSUM") as psT, \
             tc.tile_pool(name="pB1", bufs=1) as pB1, \
             tc.tile_pool(name="psP", bufs=1, space="PSUM") as psP:

            # ---- phase A: front-end basis + combine, both chunks -------
            for c in range(2):
                xc = x[:, c:c + 3:2, :]                       # (128,2,1024)
                h = fep.tile([128, 2, 1024], F16, tag="h", name=f"h{c}")
                nc.scalar.activation(h[:], xc, AF.Tanh, scale=0.5)
                bs = {nm: fep.tile([128, 2, 1024], F16, tag=nm,
                                   name=f"{nm}{c}") for nm in BAS}
                basis[c] = bs
                nc.scalar.activation(bs["sA"][:], h[:], AF.Sin, scale=PI)
                sB = fep.tile([128, 2, 1024], F16, tag="sB", name=f"sB{c}")
                nc.scalar.activation(sB[:], h[:], AF.Sin, scale=PI / 2)
                nc.scalar.activation(sB[:], sB[:], AF.Square)        # u
                nc.vector.tensor_scalar(bs["c1"][:], sB[:], -2.0, 1.0,
                                        ALU.mult, ALU.add)
                nc.scalar.activation(bs["p_"][:], bs["sA"][:], AF.Square)
                nc.vector.tensor_tensor(bs["m"][:], bs["sA"][:],
                                        bs["c1"][:], ALU.mult)
                nc.vector.tensor_tensor(bs["sp"][:], bs["sA"][:],
                                        bs["p_"][:], ALU.mult)
                nc.vector.tensor_tensor(bs["cp"][:], bs["c1"][:],
                                        bs["p_"][:], ALU.mult)
                nc.vector.tensor_tensor(bs["mp"][:], bs["m"][:],
                                        bs["p_"][:], ALU.mult)
                nc.scalar.activation(bs["mm"][:], bs["m"][:], AF.Square)

                for pc in range(4):
                    rs, c0 = pc // 2, 512 * (pc % 2)
                    pss = [psA.tile([128, 512], F32, tag=f"c{i}",
                                    name=f"c{i}") for i in range(3)]
                    for f in range(8):
                        mv = bs[BAS[f]][:, rs, c0:c0 + 512]
                        for i in range(3):
                            nc.tensor.matmul(
                                pss[i][:], combw[:, 8 * i + f, :], mv,
                                start=(f == 0), stop=(f == 7))
                    dst = [qkv[c][i][:, rs, c0:c0 + 512] for i in range(3)]
                    nc.scalar.activation(dst[0], pss[0][:], AF.Identity,
                                         bias=kvec[:, 0:1])
                    if c == 0:
                        nc.vector.tensor_scalar(dst[1], pss[1][:],
                                                kvec[:, 1:2], None, ALU.add)
                    else:
                        nc.scalar.activation(dst[1], pss[1][:], AF.Identity,
                                             bias=kvec[:, 1:2])
                    nc.scalar.activation(dst[2], pss[2][:], AF.Identity,
                                         bias=kvec[:, 2:3])

            def _warm(n, tag):
                jp = psP.tile([128, 512], F32, tag="pa",
                              name=f"warm{tag}")
                for j in range(n):
                    nc.tensor.matmul(jp[:], combw[:, j % 24, :],
                                     aftw[:, j % 8, 0:512],
                                     start=(j == 0), stop=(j == n - 1))
            _warm(0, "a")

            # ---- phase B+C: rms+rotary+AFT, proj, A2A#1, per chunk -----
            for c in range(2):
                # Per tensor: stats (rotation-invariant) run concurrent
                # with the in-place rotary on the other engine; the rms
                # apply follows both (rot(x)*rf == rot(x*rf)).
                cb = rotc1[:].unsqueeze(1).broadcast_to([128, 2, 16, 16])
                sb_ = rots1[:].unsqueeze(1).broadcast_to([128, 2, 16, 16])
                sq = pB.tile([128, 2, 1024], F16, tag="sq", name=f"sq{c}")
                rfs = [None, None, None]

                def _stats(i):
                    if c == 0:
                        nc.vector.tensor_tensor(
                            sq[:].rearrange("p a b -> p (a b)"),
                            qkv[c][i][:].rearrange("p a b -> p (a b)"),
                            qkv[c][i][:].rearrange("p a b -> p (a b)"),
                            ALU.mult)
                    else:
                        nc.scalar.activation(
                            sq[:].rearrange("p a b -> p (a b)"),
                            qkv[c][i][:].rearrange("p a b -> p (a b)"),
                            AF.Square)
                    ssq = pB.tile([128, 32], F32, tag=f"ssq{i}",
                                  name=f"ssq{i}_{c}")
                    nc.vector.tensor_reduce(
                        ssq[:],
                        sq[:].rearrange("p a (h d) -> p (a h) d", h=16),
                        axis=mybir.AxisListType.X, op=ALU.add)
                    nc.vector.tensor_scalar(ssq[:], ssq[:], 1.0 / 64, EPS,
                                            ALU.mult, ALU.add)
                    rf = _quake_rsqrt(nc, pB, ssq[:], 32, f"rf{i}",
                                      f"rf{i}_{c}")
                    rf16 = pB.tile([128, 32], F16, tag=f"rh{i}",
                                   name=f"rh{i}_{c}")
                    nc.vector.tensor_copy(rf16[:], rf[:])
                    rfs[i] = rf16

                def _rot(i, eng):
                    qv = qkv[c][i][:].rearrange("p a (h d) -> p a h d",
                                                h=16)
                    x1 = qv[:, :, :, 0:16]
                    x2 = qv[:, :, :, 32:48]
                    u1 = pB.tile([128, 2, 16, 16], F16, tag="ru1",
                                 name=f"ru1{i}_{c}")
                    u2 = pB.tile([128, 2, 16, 16], F16, tag="ru2",
                                 name=f"ru2{i}_{c}")
                    t1 = pB.tile([128, 2, 16, 16], F16, tag="rt1",
                                 name=f"rt1{i}_{c}")
                    eng.tensor_tensor(u1[:], x2, sb_, ALU.mult)
                    eng.tensor_tensor(u2[:], x1, sb_, ALU.mult)
                    eng.tensor_tensor(t1[:], x1, cb, ALU.mult)
                    eng.tensor_tensor(x1, t1[:], u1[:], ALU.add)
                    eng.tensor_tensor(t1[:], x2, cb, ALU.mult)
                    eng.tensor_tensor(x2, t1[:], u2[:], ALU.subtract)

                def _apply(i, eng):
                    rb = rfs[i][:].rearrange("p (a h) -> p a h", a=2) \
                        .unsqueeze(3).broadcast_to([128, 2, 16, 64])
                    v4 = qkv[c][i][:].rearrange("p a (h d) -> p a h d",
                                                h=16)
                    eng.tensor_tensor(v4, v4, rb, ALU.mult)

                _stats(0)
                _stats(1)
                _stats(2)
                _apply(0, nc.vector)
                _apply(1, nc.gpsimd)
                _apply(2, nc.vector)
                _rot(0, nc.vector)
                _rot(1, nc.gpsimd)

                ek = qkv[c][1]
                nc.scalar.activation(ek[:].rearrange("p a b -> p (a b)"),
                                     ek[:].rearrange("p a b -> p (a b)"),
                                     AF.Exp)
                s_ = pB.tile([128, 1024], F32, tag="s_", name=f"s_{c}")
                nc.vector.tensor_tensor(s_[:], ek[:, 0, :], ek[:, 1, :],
                                        ALU.add)
                sinv = pB1.tile([128, 1024], F32, tag="sinv",
                               name=f"sinv{c}")
                nc.vector.reciprocal_approx_fast(sinv[:], s_[:])
                t0 = pB.tile([128, 1024], F16, tag="t0", name=f"t0{c}")
                t1_ = pB.tile([128, 1024], F16, tag="t1", name=f"t1{c}")
                nc.vector.tensor_tensor(t0[:], ek[:, 0, :],
                                        qkv[c][2][:, 0, :], ALU.mult)
                nc.gpsimd.tensor_tensor(t1_[:], ek[:, 1, :],
                                        qkv[c][2][:, 1, :], ALU.mult)
                nc.vector.tensor_tensor(t0[:], t0[:], t1_[:], ALU.add)
                r_ = pB.tile([128, 1024], F16, tag="r_", name=f"r_{c}")
                nc.vector.tensor_tensor(r_[:], t0[:], sinv[:], ALU.mult)
                tq = qkv[c][0]
                nc.scalar.activation(tq[:].rearrange("p a b -> p (a b)"),
                                     tq[:].rearrange("p a b -> p (a b)"),
                                     AF.Tanh, scale=0.5)
                y1 = qkv[c][2]
                rb_ = r_[:].unsqueeze(1).broadcast_to([128, 2, 1024])
                nc.vector.scalar_tensor_tensor(
                    y1[:], tq[:], 1.0, rb_, ALU.add, ALU.mult)

                # transpose y1 -> (c-part, t-free), aft projection
                y1T = pB.tile([128, 8, 256], F16, tag="y1T", name=f"y1T{c}")
                for rs in range(2):
                    for cp2 in range(4):
                        pst = psT.tile([128, 256], F16, tag="pst",
                                       name="pst")
                        for k2 in range(2):
                            cb8 = 2 * cp2 + k2
                            nc.tensor.transpose(
                                pst[:, 128 * k2:128 * (k2 + 1)],
                                y1[:, rs, 128 * cb8:128 * (cb8 + 1)],
                                ident[:])
                        nc.any.tensor_copy(
                            y1T[:, 2 * cp2:2 * cp2 + 2,
                                128 * rs:128 * (rs + 1)],
                            pst[:].rearrange("p (a b) -> p a b", a=2))
                for rs in range(2):
                    for oc in range(2):
                        pa = psP.tile([128, 512], F32, tag="pa", name="pa")
                        for cb8 in range(8):
                            nc.tensor.matmul(
                                pa[:],
                                y1T[:, cb8, 128 * rs:128 * (rs + 1)],
                                aftw[:, cb8, 512 * oc:512 * (oc + 1)],
                                start=(cb8 == 0), stop=(cb8 == 7))
                        nc.any.tensor_copy(
                            a_ch[c][:, rs, 512 * oc:512 * (oc + 1)],
                            pa[:])
                for b in range(2):
                    nc.sync.dma_start(
                        out=a2a1_in[b][:, c, :, :].rearrange(
                            "d p c2 -> p d c2"),
                        in_=a_ch[c][:, b, :].rearrange(
                            "p (d c2) -> p d c2", d=8))
                _warm(0 if c == 0 else 40, f"c{c}")
            nc.gpsimd.collective_compute(
                "AllToAll", ALU.bypass,
                replica_groups=[list(range(NCORES))],
                ins=[dum_in[:].opt()], outs=[dum_out[:].opt()])
            nc.gpsimd.collective_compute(
                "AllToAll", ALU.bypass,
                replica_groups=[list(range(NCORES))],
                ins=[dum_in[:].opt()], outs=[dum_out[:].opt()])
            for b in range(2):
                nc.gpsimd.collective_compute(
                    "AllToAll", ALU.bypass,
                    replica_groups=[list(range(NCORES))],
                    ins=[a2a1_in[b][:].opt()],
                    outs=[a2a1_out[b][:].opt()])

        # ============ stage 2: causal SDPA + stage 3 projection =========
        with tc.tile_pool(name="pE", bufs=1) as pE, \
             tc.tile_pool(name="psE", bufs=2, space="PSUM") as psE, \
             tc.tile_pool(name="psY", bufs=1, space="PSUM") as psY, \
             tc.tile_pool(name="psQ", bufs=1, space="PSUM") as psQ, \
             tc.tile_pool(name="pe16", bufs=6) as pe16, \
             tc.tile_pool(name="pH", bufs=1) as pH, \
             tc.tile_pool(name="psH", bufs=1, space="PSUM") as psH:
            out_sb = pH.tile([128, 4, 1024], F32, tag="out_sb",
                             name="out_sb")
            mhaw = pH.tile([128, 8, 1024], F16, tag="mhaw", name="mhaw")
            nc.scalar.dma_start(out=mhaw[:], in_=mhaT_d[:])
            for b in range(2):
                A2 = pE.tile([128, 16, 128], F16, tag=f"A2_{b}",
                             name=f"A2_{b}")
                nc.sync.dma_start(
                    out=A2[:],
                    in_=a2a1_out[b][:].rearrange(
                        "s c p c2 -> p (s c) c2"))
                sq2 = pE.tile([128, 16, 128], F32, tag="sq2",
                              name=f"sq2_{b}")
                nc.scalar.activation(
                    sq2[:].rearrange("p s c2 -> p (s c2)"),
                    A2[:].rearrange("p s c2 -> p (s c2)"), AF.Square)
                ssq2 = pE.tile([128, 32], F32, tag="ssq2", name=f"ssq2{b}")
                nc.vector.tensor_reduce(
                    ssq2[:],
                    sq2[:].rearrange("p s (h d) -> p (s h) d", h=2),
                    axis=mybir.AxisListType.X, op=ALU.add)
                nc.vector.tensor_scalar(ssq2[:], ssq2[:], 1.0 / 64, EPS,
                                        ALU.mult, ALU.add)
                rf2 = _quake_rsqrt(nc, pE, ssq2[:], 32, f"rf2_{b}")
                rf216 = pE.tile([128, 32], F16, tag="rf216",
                                name=f"rf216_{b}")
                nc.vector.tensor_copy(rf216[:], rf2[:])
                V16 = pE.tile([128, 16, 2, 65], BF16, tag=f"V16_{b}",
                              name=f"V16_{b}")
                A16 = pE.tile([128, 16, 128], F16, tag=f"A16_{b}",
                              name=f"A16_{b}")
                rb2 = rf216[:].rearrange("p (s h) -> p s h", s=16) \
                    .unsqueeze(3).broadcast_to([128, 16, 2, 64])
                nc.vector.tensor_tensor(
                    A16[:].rearrange("p s (h d) -> p s h d", h=2),
                    A2[:].rearrange("p s (h d) -> p s h d", h=2),
                    rb2, ALU.mult)
                nc.vector.tensor_copy(
                    V16[:, :, :, 0:64],
                    A16[:].rearrange("p s (h d) -> p s h d", h=2))
                nc.vector.memset(V16[:, :, :, 64:65], 1.0)

                QT = pE.tile([128, 2048], F16, tag=f"QT_{b}",
                             name=f"QT_{b}")
                for jq in range(4):
                    pst2 = psQ.tile([128, 512], F16, tag="pst2",
                                    name="pst2")
                    for k4 in range(4):
                        j = 4 * jq + k4
                        nc.tensor.transpose(
                            pst2[:, 128 * k4:128 * (k4 + 1)],
                            A16[:, j, :], ident[:])
                    nc.any.tensor_copy(QT[:, 512 * jq:512 * (jq + 1)],
                                       pst2[:])

                Y16 = pE.tile([64, 8, 512], F16, tag=f"Y16_{b}",
                              name=f"Y16_{b}")
                for qc in range(4):
                    pys = [psY.tile([65, 512], F32, tag=f"py{hh}",
                                    name=f"py{hh}") for hh in range(2)]
                    nkt = 4 * qc + 4
                    for kt in range(nkt):
                        off = max(0, 128 * (kt - 4 * qc))
                        ps_s = psE.tile([128, 2, 512], F32, tag="ps_s",
                                        name="ps_s")
                        for hh in range(2):
                            hb = 64 * hh
                            nc.tensor.matmul(
                                ps_s[:, hh, off:512],
                                QT[hb:hb + 64, 128 * kt:128 * (kt + 1)],
                                QT[hb:hb + 64, 512 * qc + off:
                                   512 * (qc + 1)],
                                start=True, stop=True)
                        e16 = pe16.tile([128, 2, 512], BF16, tag="e16",
                                        name="e16")
                        if kt % 2 == 1:
                            # Schraudolph exp in bf16 bit-space on DVE:
                            # bits = round(S*0.12*(2^7/ln2) + 127*128 - 11.5)
                            nc.vector.tensor_scalar(
                                e16[:, :, off:512].bitcast(I16),
                                ps_s[:, :, off:512],
                                22.159803, 16245.5, ALU.mult, ALU.add)
                        else:
                            nc.scalar.activation(e16[:, :, off:512],
                                                 ps_s[:, :, off:512], AF.Exp,
                                                 scale=SDPA_SCALE)
                        if kt >= 4 * qc:
                            for hh in range(2):
                                eh = e16[:, hh, off:off + 128]
                                nc.gpsimd.affine_select(
                                    out=eh, in_=eh,
                                    compare_op=ALU.is_ge, fill=0.0,
                                    base=0, pattern=[[1, 128]],
                                    channel_multiplier=-1)
                        for hh in range(2):
                            nc.tensor.matmul(
                                pys[hh][:, off:512],
                                V16[:, kt, hh, :],
                                e16[:, hh, off:512],
                                start=(kt == 0), stop=(kt == nkt - 1))
                    for hh in range(2):
                        den = pE.tile([1, 512], F32, tag="den",
                                      name=f"den{b}{qc}{hh}")
                        nc.any.tensor_copy(den[:], pys[hh][64:65, :])
                        rinv = pE.tile([1, 512], F32, tag="rinv",
                                       name=f"rinv{b}{qc}{hh}")
                        nc.vector.reciprocal_approx_fast(rinv[:], den[:])
                        RSR = pE.tile([64, 512], F32, tag=f"RSR{hh}",
                                      name=f"RSR{hh}")
                        nc.gpsimd.partition_broadcast(RSR[:], rinv[:])
                        nc.vector.tensor_tensor(
                            Y16[:, 4 * hh + qc, :], pys[hh][0:64, :],
                            RSR[:], ALU.mult)
                        nc.sync.dma_start(
                            out=a2a2_in[b][2 * qc:2 * qc + 2, hh, :, :]
                                .rearrange("d p q -> p d q"),
                            in_=Y16[:, 4 * hh + qc, :].rearrange(
                                "p (d q) -> p d q", d=2))
                nc.gpsimd.collective_compute(
                    "AllToAll", ALU.bypass,
                    replica_groups=[list(range(NCORES))],
                    ins=[a2a2_in[b][:].opt()],
                    outs=[a2a2_out[b][:].opt()])

            # ---- stage 3: output projection, per batch -----------------
            for b in range(2):
                YF = pH.tile([128, 8, 256], F16, tag=f"YF{b}",
                             name=f"YF{b}")
                nc.scalar.dma_start(
                    out=YF[:],
                    in_=a2a2_out[b][:].rearrange(
                        "s hh p q -> (hh p) s q"))
                for rl in range(2):
                    rt = 2 * b + rl
                    for oc in range(2):
                        pm = psH.tile([128, 512], F32, tag="pm", name="pm")
                        for s_ in range(NCORES):
                            nc.tensor.matmul(
                                pm[:],
                                YF[:, s_, 128 * rl:128 * (rl + 1)],
                                mhaw[:, s_, 512 * oc:512 * (oc + 1)],
                                start=(s_ == 0), stop=(s_ == 7))
                        nc.any.tensor_copy(
                            out_sb[:, rt, 512 * oc:512 * (oc + 1)], pm[:])
                nc.scalar.dma_start(out=out_d[:, 2 * b:2 * b + 2, :],
                                    in_=out_sb[:, 2 * b:2 * b + 2, :])

    nc.compile()
    return nc


def _host_inputs(x, kqv, c_proj):
    """Build per-core input maps from the full problem inputs."""
    A = kqv[:, :5].astype(np.float64)     # sin coefs (col n)
    Bc = kqv[:, 5:].astype(np.float64)    # cos coefs
    coef = np.zeros((8, 3), np.float64)   # basis {s,c1,m,p,sp,cp,mp,mm}
    K = A[:, 0] + Bc[:, 0] + Bc[:, 2] + Bc[:, 4]
    coef[0] = A[:, 1] + 3.0 * A[:, 3]
    coef[1] = Bc[:, 1] + Bc[:, 3]
    coef[2] = 2.0 * A[:, 2] + 4.0 * A[:, 4]
    coef[3] = -2.0 * Bc[:, 2]
    coef[4] = -4.0 * A[:, 3]
    coef[5] = -4.0 * Bc[:, 3]
    coef[6] = -8.0 * A[:, 4]
    coef[7] = -8.0 * Bc[:, 4]

    eye = np.eye(128, dtype=np.float32)
    combw = np.zeros((128, 24, 128), np.float16)
    for i in range(3):
        for f in range(8):
            combw[:, 8 * i + f, :] = (eye * np.float32(coef[f, i])) \
                .astype(np.float16)
    kvec = np.zeros((128, 4), np.float32)
    kvec[:, :3] = K.astype(np.float32)[None, :]

    freq = (1.0 / 1024.0) ** np.linspace(0.0, 1.0, 16, dtype=np.float32)
    hh = np.arange(16, dtype=np.float32)
    theta = np.outer(hh, freq)                       # (16 heads, 16 j)
    rotc1 = np.broadcast_to(np.cos(theta).astype(np.float16),
                            (128, 16, 16)).copy()
    rots1 = np.broadcast_to(np.sin(theta).astype(np.float16),
                            (128, 16, 16)).copy()

    W1 = c_proj[:, :C]
    W2 = c_proj[:, C:]
    aftT = (0.5 * W1.T).reshape(8, 128, 1024).transpose(1, 0, 2) \
        .astype(np.float16).copy()
    mhaT = W2.T.reshape(8, 128, 1024).transpose(1, 0, 2) \
        .astype(np.float16).copy()

    in_maps = []
    for c in range(NCORES):
        xs = x[:, TSH * c:TSH * (c + 1), :].reshape(4, 128, 1024) \
            .transpose(1, 0, 2).copy()
        in_maps.append(dict(xs=np.ascontiguousarray(xs), combw=combw,
                            kvec=kvec, rotc1=rotc1, rots1=rots1,
                            aftT=aftT, mhaT=mhaT))
    return in_maps


def kernel(x, kqv, c_proj):
    x = np.asarray(x, np.float32)
    kqv = np.asarray(kqv, np.float32)
    c_proj = np.asarray(c_proj, np.float32)
    if "nc" not in _CACHE:
        _CACHE["nc"] = build()
    nc = _CACHE["nc"]
    in_maps = _host_inputs(x, kqv, c_proj)
    res = run_bass_kernel_spmd(nc, in_maps, core_ids=list(range(NCORES)))
    out = np.empty((B, T, C), np.float32)
    for c in range(NCORES):
        oc = res.results[c]["out"]          # (128, 4, 1024)
        oc = oc.transpose(1, 0, 2).reshape(B, TSH, C)
        out[:, TSH * c:TSH * (c + 1), :] = oc
    return out


# revision 30
# speedup vs baseline: 1.1140x; 1.0119x over previous
"""nn_AttentionOnDetail Trainium2 Bass kernel, 8 NeuronCores.  v2

Sharding: stage 1 (AFT) is T-sharded (each core owns 256 timesteps of both
batches); 4 quarter-size fp16 AllToAlls re-shard to head-parallel (each
core owns 2 heads of 16) for the causal SDPA; 2 fp16 AllToAlls re-shard
back to T for the output projection.  Per-core stage-1 layout is
(128 partitions, 4 row-tiles, 1024 ch), row r = rt*128 + p
(rt 0,1 = batch 0 t-halves; rt 2,3 = batch 1).  Stage-1 is processed in
two chunks c = {rt, rt+2} (the AFT reduction pairs rt with rt+2) so the
first A2A fires at the half-way point.

Math notes:
 - xs = 2*pi*sigmoid(x) - pi == pi*tanh(x/2); harmonics sin/cos(n*pi*h)
   built from s=sin(pi*h), c1=1-2*sin(pi*h/2)^2 via Chebyshev products.
 - The 3x8 harmonic combination runs on TensorEngine as fp16 matmuls with
   scaled-identity stationary weights (PSUM accumulates); +K bias applied
   during the PSUM->SBUF writeback.
 - Stage-2 rotary is skipped entirely: q == k and the rotation is
   orthogonal per head, so it cancels in q @ k^T.
 - SDPA scores are computed transposed (S^T[k,q]); the two heads' score
   matmuls use partition bases 0/64 -> concurrent PE row-groups.  Causal
   diagonal tiles are width-trimmed (only cols >= 128*(kt-4qc) computed)
   and the residual triangle masked by a uniform gpsimd affine_select on
   the leading 128 columns.  V' = [V | 1] gives softmax denominators free.
 - Activation-table discipline: all Sin ops (both chunks) issue before the
   first Exp, so ACT swaps tables exactly once (silu -> exp set).
"""
import sys
import numpy as np

sys.path.insert(0, "/opt/trn_rl_repo")

import concourse.bass as bass
import concourse.mybir as mybir
import concourse.tile as tile
from concourse import bacc
from concourse.bass_utils import run_bass_kernel_spmd
from concourse.masks import make_identity

F32 = mybir.dt.float32
F32R = mybir.dt.float32r
F16 = mybir.dt.float16
BF16 = mybir.dt.bfloat16
I16 = mybir.dt.int16
I32 = mybir.dt.int32
AF = mybir.ActivationFunctionType
ALU = mybir.AluOpType

B, T, C, H, HD = 2, 2048, 1024, 16, 64
NCORES = 8
TSH = T // NCORES            # 256 timesteps per core
EPS = 1.1920929e-07
SDPA_SCALE = 0.12
PI = float(np.pi)

_CACHE = {}


def _quake_rsqrt(nc, pool, m_ap, n, tag, name=None):
    """rfac = 1/sqrt(m) for m (128, n) positive fp32, on DVE only."""
    name = name or tag
    sh = [128, n]
    it = pool.tile(sh, I32, tag=f"{tag}_i", name=f"{name}_i")
    nc.vector.tensor_scalar(it[:], m_ap.bitcast(I32), 1, None,
                            ALU.logical_shift_right)
    sd = pool.tile(sh, I32, tag=f"{tag}_s", name=f"{name}_s")
    nc.vector.tensor_scalar(sd[:], it[:], -1, 0x5F3759DF, ALU.mult, ALU.add)
    y0 = sd[:].bitcast(F32)
    t = pool.tile(sh, F32, tag=f"{tag}_t", name=f"{name}_t")
    nc.vector.tensor_tensor(t[:], y0, y0, ALU.mult)
    nc.vector.tensor_tensor(t[:], t[:], m_ap, ALU.mult)
    nc.vector.tensor_scalar(t[:], t[:], -0.5, 1.5, ALU.mult, ALU.add)
    nc.vector.tensor_tensor(t[:], t[:], y0, ALU.mult)       # y1
    y2 = pool.tile(sh, F32, tag=f"{tag}_y", name=f"{name}_y")
    nc.vector.tensor_tensor(y2[:], t[:], t[:], ALU.mult)
    nc.vector.tensor_tensor(y2[:], y2[:], m_ap, ALU.mult)
    nc.vector.tensor_scalar(y2[:], y2[:], -0.5, 1.5, ALU.mult, ALU.add)
    nc.vector.tensor_tensor(y2[:], y2[:], t[:], ALU.mult)
    return y2


def build():
    nc = bacc.Bacc("TRN2", target_bir_lowering=False, debug=False,
                   num_devices=NCORES)
    xs_d = nc.dram_tensor("xs", [128, 4, 1024], F32, kind="ExternalInput")
    combw_d = nc.dram_tensor("combw", [128, 24, 128], F16,
                             kind="ExternalInput")
    kvec_d = nc.dram_tensor("kvec", [128, 4], F32, kind="ExternalInput")
    rotc1_d = nc.dram_tensor("rotc1", [128, 16, 16], F16,
                             kind="ExternalInput")
    rots1_d = nc.dram_tensor("rots1", [128, 16, 16], F16,
                             kind="ExternalInput")
    aftT_d = nc.dram_tensor("aftT", [128, 8, 1024], F16,
                            kind="ExternalInput")
    mhaT_d = nc.dram_tensor("mhaT", [128, 8, 1024], F16,
                            kind="ExternalInput")
    out_d = nc.dram_tensor("out", [128, 4, 1024], F32, kind="ExternalOutput")

    with tile.TileContext(nc) as tc:
      with tc.tile_pool(name="glob", bufs=1) as gp, \
           tc.tile_pool(name="projp", bufs=1) as pw, \
           tc.tile_pool(name="dram", bufs=1, space="DRAM") as dpool:

        # ---- DRAM bounce buffers for the collectives -------------------
        a2a1_in = [dpool.tile([NCORES, 2, 128, 128], F16,
                              name=f"a2a1_in{b}") for b in range(2)]
        a2a1_out = [dpool.tile([NCORES, 2, 128, 128], F16,
                               name=f"a2a1_out{b}") for b in range(2)]
        a2a2_in = [dpool.tile([NCORES, 2, 64, 256], F16,
                              name=f"a2a2_in{b}") for b in range(2)]
        a2a2_out = [dpool.tile([NCORES, 2, 64, 256], F16,
                               name=f"a2a2_out{b}") for b in range(2)]
        dum_in = dpool.tile([8, 4], F32, name="dum_in")
        dum_out = dpool.tile([8, 4], F32, name="dum_out")

        # ---- input DMAs (x per chunk so fe(c0) starts early) -----------
        x = gp.tile([128, 4, 1024], F32, tag="x", name="x")
        for c in range(2):
            nc.sync.dma_start(out=x[:, c:c + 3:2, :],
                              in_=xs_d[:, c:c + 3:2, :])
        kvec = gp.tile([128, 4], F32, name="kvec")
        nc.sync.dma_start(out=kvec[:], in_=kvec_d[:])
        nc.sync.dma_start(out=dum_in[:], in_=kvec_d[0:8, :])
        nc.gpsimd.collective_compute(
            "AllToAll", ALU.bypass,
            replica_groups=[list(range(NCORES))],
            ins=[dum_in[:].opt()], outs=[dum_out[:].opt()])
        combw = gp.tile([128, 24, 128], F16, name="combw")
        nc.sync.dma_start(out=combw[:], in_=combw_d[:])
        rotc1 = gp.tile([128, 16, 16], F16, name="rotc1")
        rots1 = gp.tile([128, 16, 16], F16, name="rots1")
        nc.sync.dma_start(out=rotc1[:], in_=rotc1_d[:])
        nc.sync.dma_start(out=rots1[:], in_=rots1_d[:])
        aftw = pw.tile([128, 8, 1024], F16, tag="projw", name="aftw")
        nc.sync.dma_start(out=aftw[:], in_=aftT_d[:])
        ident = gp.tile([128, 128], F16, name="ident")
        make_identity(nc, ident[:])

        qkv = [[gp.tile([128, 2, 1024], F16, tag=f"qkv{i}_{c}",
                        name=f"qkv{i}_{c}") for i in range(3)]
               for c in range(2)]
        a_ch = [gp.tile([128, 2, 1024], F16, tag=f"a_{c}", name=f"a_{c}")
                for c in range(2)]

        BAS = ("sA", "c1", "m", "p_", "sp", "cp", "mp", "mm")
        basis = [dict() for _ in range(2)]

        # ================= stage 1 (both chunks) ========================
        with tc.tile_pool(name="fe", bufs=2) as fep, \
             tc.tile_pool(name="psA", bufs=2, space="PSUM") as psA, \
             tc.tile_pool(name="pB", bufs=2) as pB, \
             tc.tile_pool(name="psT", bufs=1, space="PSUM") as psT, \
             tc.tile_pool(name="pB1", bufs=1) as pB1, \
             tc.tile_pool(name="psP", bufs=1, space="PSUM") as psP:

            # ---- phase A: front-end basis + combine, both chunks -------
            for c in range(2):
                xc = x[:, c:c + 3:2, :]                       # (128,2,1024)
                h = fep.tile([128, 2, 1024], F16, tag="h", name=f"h{c}")
                nc.scalar.activation(h[:], xc, AF.Tanh, scale=0.5)
                bs = {nm: fep.tile([128, 2, 1024], F16, tag=nm,
                                   name=f"{nm}{c}") for nm in BAS}
                basis[c] = bs
                nc.scalar.activation(bs["sA"][:], h[:], AF.Sin, scale=PI)
                sB = fep.tile([128, 2, 1024], F16, tag="sB", name=f"sB{c}")
                nc.scalar.activation(sB[:], h[:], AF.Sin, scale=PI / 2)
                nc.scalar.activation(sB[:], sB[:], AF.Square)        # u
                nc.vector.tensor_scalar(bs["c1"][:], sB[:], -2.0, 1.0,
                                        ALU.mult, ALU.add)
                nc.scalar.activation(bs["p_"][:], bs["sA"][:], AF.Square)
                nc.vector.tensor_tensor(bs["m"][:], bs["sA"][:],
                                        bs["c1"][:], ALU.mult)
                nc.vector.tensor_tensor(bs["sp"][:], bs["sA"][:],
                                        bs["p_"][:], ALU.mult)
                nc.vector.tensor_tensor(bs["cp"][:], bs["c1"][:],
                                        bs["p_"][:], ALU.mult)
                nc.vector.tensor_tensor(bs["mp"][:], bs["m"][:],
                                        bs["p_"][:], ALU.mult)
                nc.scalar.activation(bs["mm"][:], bs["m"][:], AF.Square)

                for pc in range(4):
                    rs, c0 = pc // 2, 512 * (pc % 2)
                    pss = [psA.tile([128, 512], F32, tag=f"c{i}",
                                    name=f"c{i}") for i in range(3)]
                    for f in range(8):
                        mv = bs[BAS[f]][:, rs, c0:c0 + 512]
                        for i in range(3):
                            nc.tensor.matmul(
                                pss[i][:], combw[:, 8 * i + f, :], mv,
                                start=(f == 0), stop=(f == 7))
                    dst = [qkv[c][i][:, rs, c0:c0 + 512] for i in range(3)]
                    nc.scalar.activation(dst[0], pss[0][:], AF.Identity,
                                         bias=kvec[:, 0:1])
                    if c == 0:
                        nc.vector.tensor_scalar(dst[1], pss[1][:],
                                                kvec[:, 1:2], None, ALU.add)
                    else:
                        nc.scalar.activation(dst[1], pss[1][:], AF.Identity,
                                             bias=kvec[:, 1:2])
                    nc.scalar.activation(dst[2], pss[2][:], AF.Identity,
                                         bias=kvec[:, 2:3])

            def _warm(n, tag):
                jp = psP.tile([128, 512], F32, tag="pa",
                              name=f"warm{tag}")
                for j in range(n):
                    nc.tensor.matmul(jp[:], combw[:, j % 24, :],
                                     aftw[:, j % 8, 0:512],
                                     start=(j == 0), stop=(j == n - 1))
            _warm(0, "a")

            # ---- phase B+C: rms+rotary+AFT, proj, A2A#1, per chunk -----
            for c in range(2):
                # Per tensor: stats (rotation-invariant) run concurrent
                # with the in-place rotary on the other engine; the rms
                # apply follows both (rot(x)*rf == rot(x*rf)).
                cb = rotc1[:].unsqueeze(1).broadcast_to([128, 2, 16, 16])
                sb_ = rots1[:].unsqueeze(1).broadcast_to([128, 2, 16, 16])
                sq = pB.tile([128, 2, 1024], F16, tag="sq", name=f"sq{c}")
                rfs = [None, None, None]

                def _stats(i):
                    if c == 0:
                        nc.vector.tensor_tensor(
                            sq[:].rearrange("p a b -> p (a b)"),
                            qkv[c][i][:].rearrange("p a b -> p (a b)"),
                            qkv[c][i][:].rearrange("p a b -> p (a b)"),
                            ALU.mult)
                    else:
                        nc.scalar.activation(
                            sq[:].rearrange("p a b -> p (a b)"),
                            qkv[c][i][:].rearrange("p a b -> p (a b)"),
                            AF.Square)
                    ssq = pB.tile([128, 32], F32, tag=f"ssq{i}",
                                  name=f"ssq{i}_{c}")
                    nc.vector.tensor_reduce(
                        ssq[:],
                        sq[:].rearrange("p a (h d) -> p (a h) d", h=16),
                        axis=mybir.AxisListType.X, op=ALU.add)
                    nc.vector.tensor_scalar(ssq[:], ssq[:], 1.0 / 64, EPS,
                                            ALU.mult, ALU.add)
                    rf = _quake_rsqrt(nc, pB, ssq[:], 32, f"rf{i}",
                                      f"rf{i}_{c}")
                    rf16 = pB.tile([128, 32], F16, tag=f"rh{i}",
                                   name=f"rh{i}_{c}")
                    nc.vector.tensor_copy(rf16[:], rf[:])
                    rfs[i] = rf16

                def _rot(i, eng):
                    qv = qkv[c][i][:].rearrange("p a (h d) -> p a h d",
                                                h=16)
                    x1 = qv[:, :, :, 0:16]
                    x2 = qv[:, :, :, 32:48]
                    u1 = pB.tile([128, 2, 16, 16], F16, tag="ru1",
                                 name=f"ru1{i}_{c}")
                    u2 = pB.tile([128, 2, 16, 16], F16, tag="ru2",
                                 name=f"ru2{i}_{c}")
                    t1 = pB.tile([128, 2, 16, 16], F16, tag="rt1",
                                 name=f"rt1{i}_{c}")
                    eng.tensor_tensor(u1[:], x2, sb_, ALU.mult)
                    eng.tensor_tensor(u2[:], x1, sb_, ALU.mult)
                    eng.tensor_tensor(t1[:], x1, cb, ALU.mult)
                    eng.tensor_tensor(x1, t1[:], u1[:], ALU.add)
                    eng.tensor_tensor(t1[:], x2, cb, ALU.mult)
                    eng.tensor_tensor(x2, t1[:], u2[:], ALU.subtract)

                def _apply(i, eng):
                    rb = rfs[i][:].rearrange("p (a h) -> p a h", a=2) \
                        .unsqueeze(3).broadcast_to([128, 2, 16, 64])
                    v4 = qkv[c][i][:].rearrange("p a (h d) -> p a h d",
                                                h=16)
                    eng.tensor_tensor(v4, v4, rb, ALU.mult)

                _stats(0)
                _stats(1)
                _stats(2)
                _apply(0, nc.vector)
                _apply(1, nc.gpsimd)
                _apply(2, nc.vector)
                _rot(0, nc.vector)
                _rot(1, nc.gpsimd)

                ek = qkv[c][1]
                nc.scalar.activation(ek[:].rearrange("p a b -> p (a b)"),
                                     ek[:].rearrange("p a b -> p (a b)"),
                                     AF.Exp)
                s_ = pB.tile([128, 1024], F32, tag="s_", name=f"s_{c}")
                nc.vector.tensor_tensor(s_[:], ek[:, 0, :], ek[:, 1, :],
                                        ALU.add)
                sinv = pB1.tile([128, 1024], F32, tag="sinv",
                               name=f"sinv{c}")
                nc.vector.reciprocal_approx_fast(sinv[:], s_[:])
                t0 = pB.tile([128, 1024], F16, tag="t0", name=f"t0{c}")
                t1_ = pB.tile([128, 1024], F16, tag="t1", name=f"t1{c}")
                nc.vector.tensor_tensor(t0[:], ek[:, 0, :],
                                        qkv[c][2][:, 0, :], ALU.mult)
                nc.gpsimd.tensor_tensor(t1_[:], ek[:, 1, :],
                                        qkv[c][2][:, 1, :], ALU.mult)
                nc.vector.tensor_tensor(t0[:], t0[:], t1_[:], ALU.add)
                r_ = pB.tile([128, 1024], F16, tag="r_", name=f"r_{c}")
                nc.vector.tensor_tensor(r_[:], t0[:], sinv[:], ALU.mult)
                tq = qkv[c][0]
                nc.scalar.activation(tq[:].rearrange("p a b -> p (a b)"),
                                     tq[:].rearrange("p a b -> p (a b)"),
                                     AF.Tanh, scale=0.5)
                y1 = qkv[c][2]
                rb_ = r_[:].unsqueeze(1).broadcast_to([128, 2, 1024])
                nc.vector.scalar_tensor_tensor(
                    y1[:], tq[:], 1.0, rb_, ALU.add, ALU.mult)

                # transpose y1 -> (c-part, t-free), aft projection
                y1T = pB.tile([128, 8, 256], F16, tag="y1T", name=f"y1T{c}")
                for rs in range(2):
                    for cp2 in range(4):
                        pst = psT.tile([128, 256], F16, tag="pst",
                                       name="pst")
                        for k2 in range(2):
                            cb8 = 2 * cp2 + k2
                            nc.tensor.transpose(
                                pst[:, 128 * k2:128 * (k2 + 1)],
                                y1[:, rs, 128 * cb8:128 * (cb8 + 1)],
                                ident[:])
                        nc.any.tensor_copy(
                            y1T[:, 2 * cp2:2 * cp2 + 2,
                                128 * rs:128 * (rs + 1)],
                            pst[:].rearrange("p (a b) -> p a b", a=2))
                for rs in range(2):
                    for oc in range(2):
                        pa = psP.tile([128, 512], F32, tag="pa", name="pa")
                        for cb8 in range(8):
                            nc.tensor.matmul(
                                pa[:],
                                y1T[:, cb8, 128 * rs:128 * (rs + 1)],
                                aftw[:, cb8, 512 * oc:512 * (oc + 1)],
                                start=(cb8 == 0), stop=(cb8 == 7))
                        nc.any.tensor_copy(
                            a_ch[c][:, rs, 512 * oc:512 * (oc + 1)],
                            pa[:])
                for b in range(2):
                    nc.sync.dma_start(
                        out=a2a1_in[b][:, c, :, :].rearrange(
                            "d p c2 -> p d c2"),
                        in_=a_ch[c][:, b, :].rearrange(
                            "p (d c2) -> p d c2", d=8))
                _warm(0 if c == 0 else 40, f"c{c}")
            nc.gpsimd.collective_compute(
                "AllToAll", ALU.bypass,
                replica_groups=[list(range(NCORES))],
                ins=[dum_in[:].opt()], outs=[dum_out[:].opt()])
            nc.gpsimd.collective_compute(
                "AllToAll", ALU.bypass,
                replica_groups=[list(range(NCORES))],
                ins=[dum_in[:].opt()], outs=[dum_out[:].opt()])
            for b in range(2):
                nc.gpsimd.collective_compute(
                    "AllToAll", ALU.bypass,
                    replica_groups=[list(range(NCORES))],
                    ins=[a2a1_in[b][:].opt()],
                    outs=[a2a1_out[b][:].opt()])

        # ============ stage 2: causal SDPA + stage 3 projection =========
        with tc.tile_pool(name="pE", bufs=1) as pE, \
             tc.tile_pool(name="psE", bufs=2, space="PSUM") as psE, \
             tc.tile_pool(name="psY", bufs=1, space="PSUM") as psY, \
             tc.tile_pool(name="psQ", bufs=1, space="PSUM") as psQ, \
             tc.tile_pool(name="pe16", bufs=6) as pe16, \
             tc.tile_pool(name="pH", bufs=1) as pH, \
             tc.tile_pool(name="psH", bufs=1, space="PSUM") as psH:
            out_sb = pH.tile([128, 4, 1024], F32, tag="out_sb",
                             name="out_sb")
            mhaw = pH.tile([128, 8, 1024], F16, tag="mhaw", name="mhaw")
            nc.scalar.dma_start(out=mhaw[:], in_=mhaT_d[:])
            for b in range(2):
                A2 = pE.tile([128, 16, 128], F16, tag=f"A2_{b}",
                             name=f"A2_{b}")
                nc.sync.dma_start(
                    out=A2[:],
                    in_=a2a1_out[b][:].rearrange(
                        "s c p c2 -> p (s c) c2"))
                sq2 = pE.tile([128, 16, 128], F32, tag="sq2",
                              name=f"sq2_{b}")
                nc.scalar.activation(
                    sq2[:].rearrange("p s c2 -> p (s c2)"),
                    A2[:].rearrange("p s c2 -> p (s c2)"), AF.Square)
                ssq2 = pE.tile([128, 32], F32, tag="ssq2", name=f"ssq2{b}")
                nc.vector.tensor_reduce(
                    ssq2[:],
                    sq2[:].rearrange("p s (h d) -> p (s h) d", h=2),
                    axis=mybir.AxisListType.X, op=ALU.add)
                nc.vector.tensor_scalar(ssq2[:], ssq2[:], 1.0 / 64, EPS,
                                        ALU.mult, ALU.add)
                rf2 = _quake_rsqrt(nc, pE, ssq2[:], 32, f"rf2_{b}")
                rf216 = pE.tile([128, 32], F16, tag="rf216",
                                name=f"rf216_{b}")
                nc.vector.tensor_copy(rf216[:], rf2[:])
                V16 = pE.tile([128, 16, 2, 65], BF16, tag=f"V16_{b}",
                              name=f"V16_{b}")
                A16 = pE.tile([128, 16, 128], F16, tag=f"A16_{b}",
                              name=f"A16_{b}")
                rb2 = rf216[:].rearrange("p (s h) -> p s h", s=16) \
                    .unsqueeze(3).broadcast_to([128, 16, 2, 64])
                nc.vector.tensor_tensor(
                    A16[:].rearrange("p s (h d) -> p s h d", h=2),
                    A2[:].rearrange("p s (h d) -> p s h d", h=2),
                    rb2, ALU.mult)
                nc.vector.tensor_copy(
                    V16[:, :, :, 0:64],
                    A16[:].rearrange("p s (h d) -> p s h d", h=2))
                nc.vector.memset(V16[:, :, :, 64:65], 1.0)

                QT = pE.tile([128, 2048], F16, tag=f"QT_{b}",
                             name=f"QT_{b}")
                for jq in range(4):
                    pst2 = psQ.tile([128, 512], F16, tag="pst2",
                                    name="pst2")
                    for k4 in range(4):
                        j = 4 * jq + k4
                        nc.tensor.transpose(
                            pst2[:, 128 * k4:128 * (k4 + 1)],
                            A16[:, j, :], ident[:])
                    nc.any.tensor_copy(QT[:, 512 * jq:512 * (jq + 1)],
                                       pst2[:])

                Y16 = pE.tile([64, 8, 512], F16, tag=f"Y16_{b}",
                              name=f"Y16_{b}")
                for qc in range(4):
                    pys = [psY.tile([65, 512], F32, tag=f"py{hh}",
                                    name=f"py{hh}") for hh in range(2)]
                    nkt = 4 * qc + 4
                    for kt in range(nkt):
                        off = max(0, 128 * (kt - 4 * qc))
                        ps_s = psE.tile([128, 2, 512], F32, tag="ps_s",
                                        name="ps_s")
                        for hh in range(2):
                            hb = 64 * hh
                            nc.tensor.matmul(
                                ps_s[:, hh, off:512],
                                QT[hb:hb + 64, 128 * kt:128 * (kt + 1)],
                                QT[hb:hb + 64, 512 * qc + off:
                                   512 * (qc + 1)],
                                start=True, stop=True)
                        e16 = pe16.tile([128, 2, 512], BF16, tag="e16",
                                        name="e16")
                        if kt % 3 == 2:
                            # Schraudolph exp in bf16 bit-space on DVE:
                            # bits = round(S*0.12*(2^7/ln2) + 127*128 - 11.5)
                            nc.vector.tensor_scalar(
                                e16[:, :, off:512].bitcast(I16),
                                ps_s[:, :, off:512],
                                22.159803, 16245.5, ALU.mult, ALU.add)
                        else:
                            nc.scalar.activation(e16[:, :, off:512],
                                                 ps_s[:, :, off:512], AF.Exp,
                                                 scale=SDPA_SCALE)
                        if kt >= 4 * qc:
                            for hh in range(2):
                                eh = e16[:, hh, off:off + 128]
                                nc.gpsimd.affine_select(
                                    out=eh, in_=eh,
                                    compare_op=ALU.is_ge, fill=0.0,
                                    base=0, pattern=[[1, 128]],
                                    channel_multiplier=-1)
                        for hh in range(2):
                            nc.tensor.matmul(
                                pys[hh][:, off:512],
                                V16[:, kt, hh, :],
                                e16[:, hh, off:512],
                                start=(kt == 0), stop=(kt == nkt - 1))
                    for hh in range(2):
                        den = pE.tile([1, 512], F32, tag="den",
                                      name=f"den{b}{qc}{hh}")
                        nc.any.tensor_copy(den[:], pys[hh][64:65, :])
                        rinv = pE.tile([1, 512], F32, tag="rinv",
                                       name=f"rinv{b}{qc}{hh}")
                        nc.vector.reciprocal_approx_fast(rinv[:], den[:])
                        RSR = pE.tile([64, 512], F32, tag=f"RSR{hh}",
                                      name=f"RSR{hh}")
                        nc.gpsimd.partition_broadcast(RSR[:], rinv[:])
                        nc.vector.tensor_tensor(
                            Y16[:, 4 * hh + qc, :], pys[hh][0:64, :],
                            RSR[:], ALU.mult)
                        nc.sync.dma_start(
                            out=a2a2_in[b][2 * qc:2 * qc + 2, hh, :, :]
                                .rearrange("d p q -> p d q"),
                            in_=Y16[:, 4 * hh + qc, :].rearrange(
                                "p (d q) -> p d q", d=2))
                nc.gpsimd.collective_compute(
                    "AllToAll", ALU.bypass,
                    replica_groups=[list(range(NCORES))],
                    ins=[a2a2_in[b][:].opt()],
                    outs=[a2a2_out[b][:].opt()])

            # ---- stage 3: output projection, per batch -----------------
            for b in range(2):
                YF = pH.tile([128, 8, 256], F16, tag=f"YF{b}",
                             name=f"YF{b}")
                nc.scalar.dma_start(
                    out=YF[:],
                    in_=a2a2_out[b][:].rearrange(
                        "s hh p q -> (hh p) s q"))
                for rl in range(2):
                    rt = 2 * b + rl
                    for oc in range(2):
                        pm = psH.tile([128, 512], F32, tag="pm", name="pm")
                        for s_ in range(NCORES):
                            nc.tensor.matmul(
                                pm[:],
                                YF[:, s_, 128 * rl:128 * (rl + 1)],
                                mhaw[:, s_, 512 * oc:512 * (oc + 1)],
                                start=(s_ == 0), stop=(s_ == 7))
                        nc.any.tensor_copy(
                            out_sb[:, rt, 512 * oc:512 * (oc + 1)], pm[:])
                nc.scalar.dma_start(out=out_d[:, 2 * b:2 * b + 2, :],
                                    in_=out_sb[:, 2 * b:2 * b + 2, :])

    nc.compile()
    return nc


def _host_inputs(x, kqv, c_proj):
    """Build per-core input maps from the full problem inputs."""
    A = kqv[:, :5].astype(np.float64)     # sin coefs (col n)
    Bc = kqv[:, 5:].astype(np.float64)    # cos coefs
    coef = np.zeros((8, 3), np.float64)   # basis {s,c1,m,p,sp,cp,mp,mm}
    K = A[:, 0] + Bc[:, 0] + Bc[:, 2] + Bc[:, 4]
    coef[0] = A[:, 1] + 3.0 * A[:, 3]
    coef[1] = Bc[:, 1] + Bc[:, 3]
    coef[2] = 2.0 * A[:, 2] + 4.0 * A[:, 4]
    coef[3] = -2.0 * Bc[:, 2]
    coef[4] = -4.0 * A[:, 3]
    coef[5] = -4.0 * Bc[:, 3]
    coef[6] = -8.0 * A[:, 4]
    coef[7] = -8.0 * Bc[:, 4]

    eye = np.eye(128, dtype=np.float32)
    combw = np.zeros((128, 24, 128), np.float16)
    for i in range(3):
        for f in range(8):
            combw[:, 8 * i + f, :] = (eye * np.float32(coef[f, i])) \
                .astype(np.float16)
    kvec = np.zeros((128, 4), np.float32)
    kvec[:, :3] = K.astype(np.float32)[None, :]

    freq = (1.0 / 1024.0) ** np.linspace(0.0, 1.0, 16, dtype=np.float32)
    hh = np.arange(16, dtype=np.float32)
    theta = np.outer(hh, freq)                       # (16 heads, 16 j)
    rotc1 = np.broadcast_to(np.cos(theta).astype(np.float16),
                            (128, 16, 16)).copy()
    rots1 = np.broadcast_to(np.sin(theta).astype(np.float16),
                            (128, 16, 16)).copy()

    W1 = c_proj[:, :C]
    W2 = c_proj[:, C:]
    aftT = (0.5 * W1.T).reshape(8, 128, 1024).transpose(1, 0, 2) \
        .astype(np.float16).copy()
    mhaT = W2.T.reshape(8, 128, 1024).transpose(1, 0, 2) \
        .astype(np.float16).copy()

    in_maps = []
    for c in range(NCORES):
        xs = x[:, TSH * c:TSH * (c + 1), :].reshape(4, 128, 1024) \
            .transpose(1, 0, 2).copy()
        in_maps.append(dict(xs=np.ascontiguousarray(xs), combw=combw,
                            kvec=kvec, rotc1=rotc1, rots1=rots1,
                            aftT=aftT, mhaT=mhaT))
    return in_maps


def kernel(x, kqv, c_proj):
    x = np.asarray(x, np.float32)
    kqv = np.asarray(kqv, np.float32)
    c_proj = np.asarray(c_proj, np.float32)
    if "nc" not in _CACHE:
        _CACHE["nc"] = build()
    nc = _CACHE["nc"]
    in_maps = _host_inputs(x, kqv, c_proj)
    res = run_bass_kernel_spmd(nc, in_maps, core_ids=list(range(NCORES)))
    out = np.empty((B, T, C), np.float32)
    for c in range(NCORES):
        oc = res.results[c]["out"]          # (128, 4, 1024)
        oc = oc.transpose(1, 0, 2).reshape(B, TSH, C)
        out[:, TSH * c:TSH * (c + 1), :] = oc
    return out
